# revision 15
# baseline (speedup 1.0000x reference)
"""Trainium2 Bass kernel for DeepConvWeigthNet — fully-fused row-packed v2.

Data-parallel across 8 NeuronCores: each core processes one 512x512 image.

Phase A: all four convs fused in ONE software-pipelined loop, SBUF-resident
(no body spills to HBM). Matmuls pack output rows into M=128:
  conv1 1->32:  M=128 = 4 rows x 32ch, K=rows of x (accum 3 kx shifts)
  conv2 32->64: M=128 = 2 rows x 64ch, K=128 = 4 body1 rows x 32ch
  conv3 64->32: M=128 = 4 rows x 32ch, K=128 = 2 body2 rows x 64ch (3 chunks)
  heads 32->12: M=96  = 8 rows x 12ch, K=128 = 4 body3 rows x 32ch (3 chunks)
Activation evictions write each conv's output directly in the NEXT conv's
rhs layout (circular SBUF buffers); only conv1 needs 2 Pool half-copies per
4 rows to build the even-window replicas.

Phase B (row-blocked layout [128 rows, 4 blocks * cols]): CA gating, channel
softmax, multiscale box blurs (DVE shift-tree along W, banded matmuls along
H), weighted combines out1 -> out2 -> out3.
"""

import os
import sys

sys.path.insert(0, "/opt/trn_rl_repo")

import numpy as np

H = W = 512
PW = 514          # padded width for all SBUF line buffers
NCORES = 8
NPIX = float(H * W)
BS = 560          # phase-B padded block stride
DOFF = 12         # phase-B data col offset within block

NG1 = H // 4      # conv1/conv3 groups (128)
NT = H // 2 + 1   # conv2 windows / V chunks (257)
NGH = 63          # heads full groups, rows 8i+1..8i+8
NQB = 7           # bottom edge rows 505..511

NWO = 6           # circular slots: W_odd, W_even, V, U
NWE = 7
NV = 6
NU = 10

DEBUG = bool(int(os.environ.get("KBENCH_DEBUG", "0")))
STAGES = os.environ.get("KBENCH_STAGES", "AB")

_CACHE = {}


# --------------------------------------------------------------------------
# host-side weight packing
# --------------------------------------------------------------------------

def _pack_host(inputs):
    f = np.float32
    w1 = np.asarray(inputs["w1"], f)   # [32,1,3,3]
    w2 = np.asarray(inputs["w2"], f)   # [64,32,3,3]
    w3 = np.asarray(inputs["w3"], f)   # [32,64,3,3]
    hw_all = np.concatenate(
        [np.asarray(inputs[f"hw{i}"], f) for i in (1, 2, 3)], axis=0)

    # conv1 lhsT variants: v=0 interior (xrep rows 4g-1..4g+4),
    # v=1 top g=0 (xrep rows 0..5), v=2 bot g=127 (xrep rows 506..511)
    w1L = np.zeros((9, 6, 128), f)
    for b in range(3):
        for j in range(6):
            for q in range(4):
                for v, aa in ((0, j - q), (1, j - q + 1), (2, j - q - 1)):
                    if 0 <= aa <= 2:
                        w1L[3 * v + b, j, 32 * q:32 * q + 32] = \
                            w1[:, 0, aa, b]

    w2L = np.zeros((3, 128, 128), f)
    for b in range(3):
        for j in range(4):
            for u in range(2):
                a = j - u
                if 0 <= a <= 2:
                    w2L[b, 32 * j:32 * j + 32, 64 * u:64 * u + 64] = \
                        w2[:, :, a, b].T

    w3L = np.zeros((9, 128, 128), f)   # index 3c+b
    for c in range(3):
        for b in range(3):
            for j in range(2):
                for q in range(4):
                    a = 2 * c + j - q
                    if 0 <= a <= 2:
                        w3L[3 * c + b, 64 * j:64 * j + 64,
                            32 * q:32 * q + 32] = w3[:, :, a, b].T

    hwL = np.zeros((9, 128, 96), f)    # index 3c+b
    for c in range(3):
        for b in range(3):
            for j in range(4):
                for r in range(8):
                    a = 4 * c + j - r
                    if 0 <= a <= 2:
                        hwL[3 * c + b, 32 * j:32 * j + 32,
                            12 * r:12 * r + 12] = hw_all[:, :, a, b].T

    hwE = np.zeros((3, 64, 12), f)
    for b in range(3):
        for j in range(2):
            hwE[b, 32 * j:32 * j + 32, :] = hw_all[:, :, j + 1, b].T

    hwB = np.zeros((6, 128, 12 * NQB), f)   # index 3c+b, c=0,1
    base0 = H - 8
    for c in range(2):
        for b in range(3):
            for j in range(4):
                for q in range(NQB):
                    a = (base0 + 4 * c + j) - (H - NQB + q) + 1
                    if 0 <= a <= 2:
                        hwB[3 * c + b, 32 * j:32 * j + 32,
                            12 * q:12 * q + 12] = hw_all[:, :, a, b].T

    sel = np.zeros((96, 12), f)
    for r in range(8):
        for c in range(12):
            sel[12 * r + c, c] = 1.0

    def blockdiag(ws):
        m = np.zeros((12, 12), f)
        for i, wca in enumerate(ws):
            m[4 * i:4 * i + 4, 4 * i:4 * i + 4] = wca[:, :, 0, 0].T
        return m

    caA = blockdiag([np.asarray(inputs[f"ca{i}a"], f) for i in (1, 2, 3)]) \
        / NPIX
    caB = blockdiag([np.asarray(inputs[f"ca{i}b"], f) for i in (1, 2, 3)])

    # banded along-H blur matrices (same as baseline)
    ks = (5, 15, 25)
    bandH = np.zeros((3, 4, 3, 128, 128), f)
    for kidx, k in enumerate(ks):
        c = (k - 1) // 2
        inv = 1.0 / (k * k)
        for t in range(4):
            for relidx, rel in enumerate((-1, 0, 1)):
                tp = t + rel
                if tp < 0 or tp > 3:
                    continue
                ii = np.arange(128)[:, None] + 128 * tp
                jj = np.arange(128)[None, :] + 128 * t
                bandH[kidx, t, relidx][np.abs(ii - jj) <= c] = inv
    bandP = np.ascontiguousarray(
        np.transpose(bandH, (3, 0, 1, 2, 4)).reshape(128, 36 * 128))

    b1 = np.asarray(inputs["b1"], f)
    b2 = np.asarray(inputs["b2"], f)
    b3 = np.asarray(inputs["b3"], f)
    hb = np.concatenate([np.asarray(inputs[f"hb{i}"], f)
                         for i in (1, 2, 3)])
    return dict(
        w1L=w1L, w2L=w2L, w3L=w3L, hwL=hwL, hwE=hwE, hwB=hwB,
        sel=sel, caA=caA, caB=caB, bandP=bandP,
        b1x4=np.tile(b1, 4).reshape(128, 1),
        b2x2=np.tile(b2, 2).reshape(128, 1),
        b3x4=np.tile(b3, 4).reshape(128, 1),
        hbx8=np.tile(hb, 8).reshape(96, 1),
        hbx7=np.tile(hb, NQB).reshape(84, 1),
        hb=hb.reshape(12, 1),
    )


# --------------------------------------------------------------------------
# kernel build
# --------------------------------------------------------------------------

def _build(alpha1, alpha2, alpha3, debug=False, loop_reps=0, stages="AB"):
    import concourse.bacc as bacc
    import concourse.mybir as mybir
    import concourse.tile as tile

    dt = mybir.dt
    AFT = mybir.ActivationFunctionType
    f32 = dt.float32
    f32r = dt.float32r

    nc = bacc.Bacc("TRN2", target_bir_lowering=False, debug=False,
                   num_devices=NCORES)

    # ---- I/O ----
    xb = nc.dram_tensor("xb", [H, W], f32, kind="ExternalInput")
    w1L_d = nc.dram_tensor("w1L", [9, 6, 128], f32, kind="ExternalInput")
    w2L_d = nc.dram_tensor("w2L", [3, 128, 128], f32, kind="ExternalInput")
    w3L_d = nc.dram_tensor("w3L", [9, 128, 128], f32, kind="ExternalInput")
    hwL_d = nc.dram_tensor("hwL", [9, 128, 96], f32, kind="ExternalInput")
    hwE_d = nc.dram_tensor("hwE", [3, 64, 12], f32, kind="ExternalInput")
    hwB_d = nc.dram_tensor("hwB", [6, 128, 12 * NQB], f32,
                           kind="ExternalInput")
    sel_d = nc.dram_tensor("sel", [96, 12], f32, kind="ExternalInput")
    caA_d = nc.dram_tensor("caA", [12, 12], f32, kind="ExternalInput")
    caB_d = nc.dram_tensor("caB", [12, 12], f32, kind="ExternalInput")
    bandP_d = nc.dram_tensor("bandP", [128, 36 * 128], f32,
                             kind="ExternalInput")
    b1x4_d = nc.dram_tensor("b1x4", [128, 1], f32, kind="ExternalInput")
    b2x2_d = nc.dram_tensor("b2x2", [128, 1], f32, kind="ExternalInput")
    b3x4_d = nc.dram_tensor("b3x4", [128, 1], f32, kind="ExternalInput")
    hbx8_d = nc.dram_tensor("hbx8", [96, 1], f32, kind="ExternalInput")
    hbx7_d = nc.dram_tensor("hbx7", [84, 1], f32, kind="ExternalInput")
    hb_d = nc.dram_tensor("hb", [12, 1], f32, kind="ExternalInput")

    outb = nc.dram_tensor("outb", [H, W], f32, kind="ExternalOutput")

    dbg = {}
    if debug:
        for name, shape in (("d_y", [H, 12, W]), ("d_g", [12, 1]),
                            ("d_h", [12, H, W]),
                            ("d_out1", [H, W]), ("d_out2", [H, W])):
            dbg[name] = nc.dram_tensor(name, shape, f32,
                                       kind="ExternalOutput")

    with tile.TileContext(nc) as tc:
        with (
            tc.tile_pool(name="dram", bufs=1, space="DRAM") as dpool,
            tc.tile_pool(name="wsb", bufs=1) as wsb,
        ):
            y_dram = dpool.tile([H, 12, W], f32)

            # persistent weight/bias tiles
            w1sb = wsb.tile([6, 9 * 128], f32r)
            w2sb = wsb.tile([128, 3 * 128], f32r)
            w3sb = wsb.tile([128, 9 * 128], f32r)
            hwsb = wsb.tile([128, 9 * 96], f32r)
            hwEsb = wsb.tile([64, 3 * 12], f32r)
            hwBsb = wsb.tile([128, 6 * 12 * NQB], f32r)
            selsb = wsb.tile([96, 12], f32)
            caAsb = wsb.tile([12, 12], f32)
            caBsb = wsb.tile([12, 12], f32)
            bandsb = wsb.tile([128, 36 * 128], f32r)
            b1sb = wsb.tile([128, 1], f32)
            b2sb = wsb.tile([128, 1], f32)
            b3sb = wsb.tile([128, 1], f32)
            hb8sb = wsb.tile([96, 1], f32)
            hb7sb = wsb.tile([84, 1], f32)
            hbsb = wsb.tile([12, 1], f32)
            onesb = wsb.tile([1, 128], f32)
            accums = wsb.tile([96, NGH + 3], f32)

            with tc.tile_pool(name="wstg", bufs=1) as wstg:
                w1f = wstg.tile([6, 9 * 128], f32)
                w2f = wstg.tile([128, 3 * 128], f32)
                w3f = wstg.tile([128, 9 * 128], f32)
                hwf = wstg.tile([128, 9 * 96], f32)
                hwEf = wstg.tile([64, 3 * 12], f32)
                hwBf = wstg.tile([128, 6 * 12 * NQB], f32)
                bandf = wstg.tile([128, 36 * 128], f32)
                for b in range(3):
                    nc.sync.dma_start(w2f[:, 128 * b:128 * (b + 1)],
                                      w2L_d[b])
                    nc.sync.dma_start(hwEf[:, 12 * b:12 * (b + 1)],
                                      hwE_d[b])
                for i in range(9):
                    nc.sync.dma_start(w1f[:, 128 * i:128 * (i + 1)],
                                      w1L_d[i])
                    nc.sync.dma_start(w3f[:, 128 * i:128 * (i + 1)],
                                      w3L_d[i])
                    nc.sync.dma_start(hwf[:, 96 * i:96 * (i + 1)],
                                      hwL_d[i])
                for i in range(6):
                    nc.sync.dma_start(hwBf[:, 84 * i:84 * (i + 1)],
                                      hwB_d[i])
                nc.sync.dma_start(bandf[:], bandP_d[:])
                nc.vector.tensor_copy(w1sb[:], w1f[:])
                nc.vector.tensor_copy(w2sb[:], w2f[:])
                nc.vector.tensor_copy(w3sb[:], w3f[:])
                nc.vector.tensor_copy(hwsb[:], hwf[:])
                nc.vector.tensor_copy(hwEsb[:], hwEf[:])
                nc.vector.tensor_copy(hwBsb[:], hwBf[:])
                nc.vector.tensor_copy(bandsb[:], bandf[:])
            nc.sync.dma_start(selsb[:], sel_d[:])
            nc.sync.dma_start(caAsb[:], caA_d[:])
            nc.sync.dma_start(caBsb[:], caB_d[:])
            nc.sync.dma_start(b1sb[:], b1x4_d[:])
            nc.sync.dma_start(b2sb[:], b2x2_d[:])
            nc.sync.dma_start(b3sb[:], b3x4_d[:])
            nc.sync.dma_start(hb8sb[:], hbx8_d[:])
            nc.sync.dma_start(hb7sb[:], hbx7_d[:])
            nc.sync.dma_start(hbsb[:], hb_d[:])
            nc.vector.memset(onesb[:], 1.0)
            nc.vector.memset(accums[:], 0.0)

            def phase_a():
                with (
                    tc.tile_pool(name="abuf", bufs=1) as abuf,
                    tc.tile_pool(name="xrp", bufs=5) as xrp,
                    tc.tile_pool(name="ystg", bufs=2) as ystg,
                    tc.tile_pool(name="ps1", bufs=2, space="PSUM") as ps1,
                    tc.tile_pool(name="ps2", bufs=3, space="PSUM") as ps2,
                    tc.tile_pool(name="ps3", bufs=2, space="PSUM") as ps3,
                    tc.tile_pool(name="psH", bufs=1, space="PSUM") as psH,
                ):
                    x_sb = abuf.tile([128, 4, PW], f32r, name="x_sb")
                    WO = abuf.tile([128, NWO, PW], f32r, name="WO")
                    WE = abuf.tile([128, NWE, PW], f32r, name="WE")
                    Vt = abuf.tile([128, NV, PW], f32r, name="Vt")
                    Ut = abuf.tile([128, NU, PW], f32r, name="Ut")

                    # x load + round to f32r + pads
                    xstg = abuf.tile([128, 4, 512], f32, name="xstg")
                    nc.sync.dma_start(
                        xstg[:], xb[:, :].rearrange("(b p) w -> p b w",
                                                    p=128))
                    nc.vector.tensor_copy(x_sb[:, :, 1:513], xstg[:])
                    nc.vector.memset(x_sb.bitcast(f32)[:, :, 0:1], 0.0)
                    nc.vector.memset(x_sb.bitcast(f32)[:, :, 513:514], 0.0)
                    # column pads of line buffers (stay zero forever)
                    for t_ in (WO, WE, Vt, Ut):
                        nc.vector.memset(t_.bitcast(f32)[:, :, 0:1], 0.0)
                        nc.vector.memset(t_.bitcast(f32)[:, :, 513:514], 0.0)
                    # boundary zeros
                    nc.vector.memset(WE.bitcast(f32)[0:64, 0, :], 0.0)
                    nc.vector.memset(Vt.bitcast(f32)[0:64, 0, :], 0.0)

                    xr_tiles = {}

                    def xrep_fetch(g):
                        # rows copied into xr partitions 0..5 via DMA
                        # (no partition-base restriction on DMA)
                        xr = xrp.tile([6, PW], f32r, tag="xrep",
                                      name="xrepT")
                        if g == 0:
                            base = 0        # rows 0..5, lhsT variant v=1
                        elif g == 127:
                            base = 506      # rows 506..511, variant v=2
                        else:
                            base = 4 * g - 1
                        r = base
                        while r <= base + 5:
                            blk = r // 128
                            rend = min(base + 5, blk * 128 + 127)
                            j0 = r - base
                            n = rend - r + 1
                            nc.sync.dma_start(
                                xr[j0:j0 + n, :],
                                x_sb[r % 128:r % 128 + n, blk, :])
                            r = rend + 1
                        xr_tiles[g] = xr

                    def conv1(g):
                        xr = xr_tiles.pop(g)
                        v = 1 if g == 0 else (2 if g == 127 else 0)
                        psum1 = ps1.tile([128, 512], f32, tag="ps1",
                                         name="ps1T")
                        for b in range(3):
                            nc.tensor.matmul(
                                psum1[:],
                                w1sb[0:6, 128 * (3 * v + b):
                                     128 * (3 * v + b) + 128],
                                xr[0:6, b:b + 512],
                                start=(b == 0), stop=(b == 2))
                        nc.scalar.activation(
                            WO[:, g % NWO, 1:513], psum1[:], AFT.Prelu,
                            bias=b1sb[:], scale=1.0, alpha=alpha1)
                        nc.gpsimd.tensor_copy(WE[64:128, g % NWE, :],
                                              WO[0:64, g % NWO, :])
                        nc.gpsimd.tensor_copy(WE[0:64, (g + 1) % NWE, :],
                                              WO[64:128, g % NWO, :])

                    def conv2(t):
                        if t % 2 == 0:
                            Wx = WE[:, (t // 2) % NWE, :]
                        else:
                            Wx = WO[:, (t // 2) % NWO, :]
                        psum2 = ps2.tile([128, 512], f32, tag="ps2",
                                         name="ps2T")
                        for b in range(3):
                            nc.tensor.matmul(
                                psum2[:],
                                w2sb[:, 128 * b:128 * b + 128],
                                Wx[:, b:b + 512],
                                start=(b == 0), stop=(b == 2))
                        if t == 0:
                            nc.scalar.activation(
                                Vt[64:128, 0, 1:513], psum2[64:128, :],
                                AFT.Prelu, bias=b2sb[64:128], scale=1.0,
                                alpha=alpha2)
                        elif t == NT - 1:
                            nc.scalar.activation(
                                Vt[0:64, t % NV, 1:513], psum2[0:64, :],
                                AFT.Prelu, bias=b2sb[0:64], scale=1.0,
                                alpha=alpha2)
                        else:
                            nc.scalar.activation(
                                Vt[:, t % NV, 1:513], psum2[:], AFT.Prelu,
                                bias=b2sb[:], scale=1.0, alpha=alpha2)

                    def conv3(h):
                        psum3 = ps3.tile([128, 512], f32, tag="ps3",
                                         name="ps3T")
                        for c in range(3):
                            for b in range(3):
                                nc.tensor.matmul(
                                    psum3[:],
                                    w3sb[:, 128 * (3 * c + b):
                                         128 * (3 * c + b) + 128],
                                    Vt[:, (2 * h + c) % NV, b:b + 512],
                                    start=(c == 0 and b == 0),
                                    stop=(c == 2 and b == 2))
                        nc.scalar.activation(
                            Ut[:, h % NU, 1:513], psum3[:], AFT.Prelu,
                            bias=b3sb[:], scale=1.0, alpha=alpha3)

                    def heads(i):
                        psumH = psH.tile([96, 512], f32, tag="psH",
                                         name="psHT")
                        for c in range(3):
                            for b in range(3):
                                nc.tensor.matmul(
                                    psumH[:],
                                    hwsb[:, 96 * (3 * c + b):
                                         96 * (3 * c + b) + 96],
                                    Ut[:, (2 * i + c) % NU, b:b + 512],
                                    start=(c == 0 and b == 0),
                                    stop=(c == 2 and b == 2))
                        stg = ystg.tile([96, 512], f32, tag="ystg",
                                        name="ystgT")
                        nc.scalar.activation(
                            stg[:], psumH[:], AFT.Prelu, bias=hb8sb[:],
                            scale=1.0, alpha=1.0,
                            accum_out=accums[:, i:i + 1])
                        nc.sync.dma_start(
                            y_dram[8 * i + 1:8 * i + 9, :, :], stg[:])

                    def heads_top():
                        psE = psH.tile([12, 512], f32, tag="psH",
                                       name="psET")
                        for b in range(3):
                            nc.tensor.matmul(
                                psE[:], hwEsb[:, 12 * b:12 * b + 12],
                                Ut[0:64, 0, b:b + 512],
                                start=(b == 0), stop=(b == 2))
                        stg = ystg.tile([12, 512], f32, tag="ystgE",
                                        name="ystgET", bufs=1)
                        nc.scalar.activation(
                            stg[:], psE[:], AFT.Prelu, bias=hbsb[:],
                            scale=1.0, alpha=1.0,
                            accum_out=accums[0:12, NGH:NGH + 1])
                        nc.sync.dma_start(y_dram[0:1, :, :], stg[:])

                    def heads_bot():
                        psB = psH.tile([84, 512], f32, tag="psH",
                                       name="psBT")
                        for c in range(2):
                            for b in range(3):
                                nc.tensor.matmul(
                                    psB[:],
                                    hwBsb[:, 84 * (3 * c + b):
                                          84 * (3 * c + b) + 84],
                                    Ut[:, (126 + c) % NU, b:b + 512],
                                    start=(c == 0 and b == 0),
                                    stop=(c == 1 and b == 2))
                        stg = ystg.tile([84, 512], f32, tag="ystgB",
                                        name="ystgBT", bufs=1)
                        nc.scalar.activation(
                            stg[:], psB[:], AFT.Prelu, bias=hb7sb[:],
                            scale=1.0, alpha=1.0,
                            accum_out=accums[0:84, NGH + 1:NGH + 2])
                        nc.sync.dma_start(y_dram[H - NQB:H, :, :], stg[:])

                    for g in range(134):
                        if g == 0:
                            xrep_fetch(0)
                            xrep_fetch(1)
                            xrep_fetch(2)
                        if g + 3 <= 127:
                            xrep_fetch(g + 3)
                        if g <= 127:
                            conv1(g)
                        if g == 128:
                            nc.vector.memset(
                                WE.bitcast(f32)[64:128, 128 % NWE, :], 0.0)
                        if 2 <= g <= 129:
                            conv2(2 * (g - 2))
                            conv2(2 * (g - 2) + 1)
                        if g == 130:
                            conv2(NT - 1)
                        if 4 <= g <= 131:
                            conv3(g - 4)
                        if g == 5:
                            heads_top()
                        if g % 2 == 1 and 7 <= g <= 131:
                            heads((g - 7) // 2)
                        if g == 133:
                            heads_bot()

            def phase_b():
                with (
                    tc.tile_pool(name="bsm", bufs=1) as bsm,
                    tc.tile_pool(name="bps1", bufs=1, space="PSUM") as bps1,
                    tc.tile_pool(name="bps", bufs=2, space="PSUM") as bps,
                    tc.tile_pool(name="bbl", bufs=1) as bbl,
                ):
                    # ---- CA gating ----
                    tot96 = bsm.tile([96, 1], f32, name="tot96")
                    nc.vector.reduce_sum(tot96[:], accums[:],
                                         axis=mybir.AxisListType.X)
                    ps12 = bps1.tile([12, 1], f32, tag="caps", name="ps12")
                    nc.tensor.matmul(ps12[:], selsb[:], tot96[:],
                                     start=True, stop=True)
                    total = bsm.tile([12, 1], f32, name="total")
                    nc.vector.tensor_copy(total[:], ps12[:])
                    psA = bps1.tile([12, 1], f32, tag="caps", name="psA")
                    nc.tensor.matmul(psA[:], caAsb[:], total[:],
                                     start=True, stop=True)
                    trelu = bsm.tile([12, 1], f32, name="trelu")
                    nc.scalar.activation(trelu[:], psA[:], AFT.Relu)
                    psB_ = bps1.tile([12, 1], f32, tag="caps", name="psB_")
                    nc.tensor.matmul(psB_[:], caBsb[:], trelu[:],
                                     start=True, stop=True)
                    g_gate = bsm.tile([12, 1], f32, name="g_gate")
                    nc.scalar.activation(g_gate[:], psB_[:], AFT.Sigmoid)
                    if debug:
                        nc.sync.dma_start(dbg["d_g"][:], g_gate[:])
                    g_row = bsm.tile([1, 12], f32, name="g_row")
                    nc.sync.dma_start(g_row[:], g_gate[:])
                    psG = bps1.tile([128, 12], f32, tag="gbc", name="psG")
                    nc.tensor.matmul(psG[:], onesb[:], g_row[:],
                                     start=True, stop=True)
                    gbc = bsm.tile([128, 12], f32, name="gbc")
                    nc.vector.tensor_copy(gbc[:], psG[:])

                    # ---- blur planes ----
                    FW = 4 * BS  # 2240
                    u = bbl.tile([128, FW], f32r, name="u")
                    S2 = bbl.tile([128, FW], f32r, name="S2")
                    S4 = bbl.tile([128, FW], f32r, name="S4")
                    S8 = bbl.tile([128, FW], f32r, name="S8")
                    S16 = bbl.tile([128, FW], f32r, name="S16")
                    S5 = bbl.tile([128, FW], f32r, name="S5")
                    S15 = bbl.tile([128, FW], f32r, name="S15")
                    S25 = bbl.tile([128, FW], f32r, name="S25")
                    unext = bbl.tile([128, FW], f32r, name="unext")
                    t1 = bbl.tile([128, 512], f32, name="t1")
                    t2 = bbl.tile([128, 512], f32, name="t2")
                    ostg = bbl.tile([128, 4, 512], f32, name="ostg")
                    nc.vector.memset(u[:].bitcast(f32), 0.0)
                    nc.vector.memset(unext[:].bitcast(f32), 0.0)

                    xt2 = bsm.tile([128, 4, 512], f32, name="xt2")
                    nc.sync.dma_start(
                        xt2[:], xb[:, :].rearrange("(b p) w -> p b w", p=128))
                    uview = u[:].rearrange("p (b w) -> p b w", b=4)
                    nc.vector.tensor_copy(uview[:, :, DOFF:DOFF + 512],
                                          xt2[:])

                    ep = [bsm.tile([128, 4, 512], f32, tag=f"exp{c}",
                                   name=f"ep{c}")
                          for c in range(4)]
                    yt = bsm.tile([128, 4, 512], f32, name="yt")
                    tsum = bsm.tile([128, 4, 512], f32, name="tsum")

                    cs = {5: 2, 15: 7, 25: 12}
                    ucur, unxt = u, unext
                    for stage in range(3):
                        for c in range(4):
                            cg = 4 * stage + c
                            nc.sync.dma_start(
                                yt[:],
                                y_dram[:, cg, :].rearrange(
                                    "(b p) w -> p b w", p=128))
                            nc.scalar.activation(ep[c][:], yt[:], AFT.Exp,
                                                 scale=gbc[:, cg:cg + 1])
                        nc.vector.tensor_add(tsum[:], ep[0][:], ep[1][:])
                        nc.vector.tensor_add(tsum[:], tsum[:], ep[2][:])
                        nc.vector.tensor_add(tsum[:], tsum[:], ep[3][:])
                        nc.vector.reciprocal(tsum[:], tsum[:])
                        for c in range(4):
                            nc.vector.tensor_mul(ep[c][:], ep[c][:],
                                                 tsum[:])
                        if debug:
                            for c in range(4):
                                nc.sync.dma_start(
                                    dbg["d_h"][4 * stage + c].rearrange(
                                        "(b p) w -> p b w", p=128),
                                    ep[c][:])

                        wv = FW - 24
                        nc.vector.tensor_add(S2[:, 0:wv], ucur[:, 0:wv],
                                             ucur[:, 1:1 + wv])
                        nc.vector.tensor_add(S4[:, 0:wv], S2[:, 0:wv],
                                             S2[:, 2:2 + wv])
                        nc.vector.tensor_add(S8[:, 0:wv], S4[:, 0:wv],
                                             S4[:, 4:4 + wv])
                        nc.vector.tensor_add(S16[:, 0:wv], S8[:, 0:wv],
                                             S8[:, 8:8 + wv])
                        nc.vector.tensor_add(S5[:, 0:wv], S4[:, 0:wv],
                                             ucur[:, 4:4 + wv])
                        nc.vector.tensor_sub(S15[:, 0:wv], S16[:, 0:wv],
                                             ucur[:, 15:15 + wv])
                        nc.vector.tensor_add(S25[:, 0:wv], S16[:, 0:wv],
                                             S8[:, 16:16 + wv])
                        nc.vector.tensor_add(S25[:, 0:wv], S25[:, 0:wv],
                                             ucur[:, 24:24 + wv])

                        Sk = {5: S5, 15: S15, 25: S25}
                        for t in range(4):
                            pk = {}
                            for kidx, k in enumerate((5, 15, 25)):
                                ps = bps.tile([128, 512], f32,
                                              tag=f"blur{kidx}",
                                              name=f"blur{kidx}")
                                rels = [r for r in (-1, 0, 1)
                                        if 0 <= t + r <= 3]
                                for ri, rel in enumerate(rels):
                                    idx = kidx * 12 + t * 3 + (rel + 1)
                                    off = (t + rel) * BS + DOFF - cs[k]
                                    nc.tensor.matmul(
                                        ps[:],
                                        bandsb[:, idx * 128:(idx + 1) * 128],
                                        Sk[k][:, off:off + 512],
                                        start=(ri == 0),
                                        stop=(ri == len(rels) - 1))
                                pk[k] = ps
                            ub = ucur[:, t * BS + DOFF:t * BS + DOFF + 512]
                            nc.vector.tensor_mul(t1[:], ep[0][:, t, :], ub)
                            nc.vector.tensor_mul(t2[:], ep[1][:, t, :],
                                                 pk[5][:])
                            nc.vector.tensor_add(t1[:], t1[:], t2[:])
                            nc.vector.tensor_mul(t2[:], ep[2][:, t, :],
                                                 pk[15][:])
                            nc.vector.tensor_add(t1[:], t1[:], t2[:])
                            nc.vector.tensor_mul(t2[:], ep[3][:, t, :],
                                                 pk[25][:])
                            if stage < 2:
                                nc.vector.tensor_add(
                                    unxt[:, t * BS + DOFF:
                                         t * BS + DOFF + 512],
                                    t1[:], t2[:])
                            else:
                                nc.vector.tensor_add(ostg[:, t, :], t1[:],
                                                     t2[:])
                        if stage < 2:
                            ucur, unxt = unxt, ucur
                            if debug:
                                dv = ucur[:].rearrange("p (b w) -> p b w",
                                                       b=4)
                                ds = bbl.tile([128, 4, 512], f32,
                                              tag="dbgo", name="dbgo")
                                nc.vector.tensor_copy(
                                    ds[:], dv[:, :, DOFF:DOFF + 512])
                                nc.sync.dma_start(
                                    dbg[f"d_out{stage + 1}"][:, :].rearrange(
                                        "(b p) w -> p b w", p=128), ds[:])

                    nc.sync.dma_start(
                        outb[:, :].rearrange("(b p) w -> p b w", p=128),
                        ostg[:])

            def phases():
                if "A" in stages:
                    phase_a()
                if debug:
                    nc.sync.dma_start(dbg["d_y"][:], y_dram[:])
                if "B" in stages:
                    phase_b()

            if loop_reps:
                with tc.For_i(0, loop_reps, 1):
                    phases()
            else:
                phases()

    nc.compile()
    return nc


# --------------------------------------------------------------------------
# PJRT runner (unchanged from baseline)
# --------------------------------------------------------------------------

class _Runner:
    def __init__(self, nc):
        import jax
        import concourse.mybir as mybir
        from concourse import bass2jax
        from jax.sharding import Mesh, PartitionSpec
        from jax.experimental.shard_map import shard_map

        bass2jax.install_neuronx_cc_hook()
        self.nc = nc
        in_names, out_names, out_avals, zero_outs = [], [], [], []
        partition_name = (nc.partition_id_tensor.name
                          if nc.partition_id_tensor else None)
        for alloc in nc.m.functions[0].allocations:
            if not isinstance(alloc, mybir.MemoryLocationSet):
                continue
            name = alloc.memorylocations[0].name
            if alloc.kind == "ExternalInput":
                if name != partition_name:
                    in_names.append(name)
            elif alloc.kind == "ExternalOutput":
                out_names.append(name)
                shape = tuple(alloc.tensor_shape)
                dtype = mybir.dt.np(alloc.dtype)
                out_avals.append(jax.core.ShapedArray(shape, dtype))
                zero_outs.append(np.zeros(shape, dtype))
        self.in_names = list(in_names)
        self.out_names = out_names
        self.out_avals = out_avals
        self.zero_outs = zero_outs
        n_params = len(in_names)
        n_outs = len(out_names)
        all_names = in_names + out_names
        if partition_name is not None:
            all_names.append(partition_name)

        def _body(*args):
            operands = list(args)
            if partition_name is not None:
                operands.append(bass2jax.partition_id_tensor())
            outs = bass2jax._bass_exec_p.bind(
                *operands,
                out_avals=tuple(out_avals),
                in_names=tuple(all_names),
                out_names=tuple(out_names),
                lowering_input_output_aliases=(),
                sim_require_finite=True,
                sim_require_nnan=True,
                nc=nc,
            )
            return tuple(outs)

        devices = jax.devices()[:NCORES]
        mesh = Mesh(np.asarray(devices), ("core",))
        in_specs = (PartitionSpec("core"),) * (n_params + n_outs)
        out_specs = (PartitionSpec("core"),) * n_outs
        self.sharded = jax.jit(
            shard_map(_body, mesh=mesh, in_specs=in_specs,
                      out_specs=out_specs, check_rep=False),
            keep_unused=True,
        )

    def concat_inputs(self, in_maps):
        return [
            np.concatenate([np.asarray(in_maps[c][nm])
                            for c in range(NCORES)], axis=0)
            for nm in self.in_names
        ]

    def concat_zeros(self):
        return [np.zeros((NCORES * z.shape[0], *z.shape[1:]), z.dtype)
                for z in self.zero_outs]

    def __call__(self, in_maps):
        out_arrs = self.sharded(*self.concat_inputs(in_maps),
                                *self.concat_zeros())
        return [
            {nm: np.asarray(out_arrs[i]).reshape(
                NCORES, *self.out_avals[i].shape)[c]
             for i, nm in enumerate(self.out_names)}
            for c in range(NCORES)
        ]


def _get_runner(alpha1, alpha2, alpha3, loop_reps=0, stages=None):
    if stages is None:
        stages = STAGES
    key = ("runner", alpha1, alpha2, alpha3, DEBUG, loop_reps, stages)
    if key not in _CACHE:
        key_nc = (alpha1, alpha2, alpha3, DEBUG, loop_reps, stages)
        if key_nc not in _CACHE:
            _CACHE[key_nc] = _build(alpha1, alpha2, alpha3, debug=DEBUG,
                                    loop_reps=loop_reps, stages=stages)
        _CACHE[key] = _Runner(_CACHE[key_nc])
    return _CACHE[key]


def make_in_maps(inputs):
    x = np.asarray(inputs["x"], np.float32)   # [8,1,512,512]
    packed = _pack_host(inputs)
    in_maps = []
    for i in range(NCORES):
        m = {"xb": np.ascontiguousarray(x[i, 0])}
        m.update(packed)
        in_maps.append(m)
    return in_maps


def kernel(**inputs):
    runner = _get_runner(float(inputs["a1"]), float(inputs["a2"]),
                         float(inputs["a3"]))
    results = runner(make_in_maps(inputs))
    out = np.stack([results[i]["outb"] for i in range(NCORES)])
    globals()["_LAST_RESULTS"] = results
    return out.reshape(8, 1, H, W).astype(np.float32)


# revision 16
# speedup vs baseline: 1.3916x; 1.3916x over previous
"""Trainium2 Bass kernel for DeepConvWeigthNet — fully-fused row-packed v2.

Data-parallel across 8 NeuronCores: each core processes one 512x512 image.

Phase A: all four convs fused in ONE software-pipelined loop, SBUF-resident
(no body spills to HBM). Matmuls pack output rows into M=128:
  conv1 1->32:  M=128 = 4 rows x 32ch, K=rows of x (accum 3 kx shifts)
  conv2 32->64: M=128 = 2 rows x 64ch, K=128 = 4 body1 rows x 32ch
  conv3 64->32: M=128 = 4 rows x 32ch, K=128 = 2 body2 rows x 64ch (3 chunks)
  heads 32->12: M=96  = 8 rows x 12ch, K=128 = 4 body3 rows x 32ch (3 chunks)
Activation evictions write each conv's output directly in the NEXT conv's
rhs layout (circular SBUF buffers); only conv1 needs 2 Pool half-copies per
4 rows to build the even-window replicas.

Phase B (row-blocked layout [128 rows, 4 blocks * cols]): CA gating, channel
softmax, multiscale box blurs (DVE shift-tree along W, banded matmuls along
H), weighted combines out1 -> out2 -> out3.
"""

import os
import sys

sys.path.insert(0, "/opt/trn_rl_repo")

import numpy as np

H = W = 512
PW = 514          # padded width for all SBUF line buffers
NCORES = 8
NPIX = float(H * W)
BS = 560          # phase-B padded block stride
DOFF = 12         # phase-B data col offset within block

NG1 = H // 4      # conv1/conv3 groups (128)
NT = H // 2 + 1   # conv2 windows / V chunks (257)
NGH = 63          # heads full groups, rows 8i+1..8i+8
NQB = 7           # bottom edge rows 505..511

NWO = 6           # circular slots: W_odd, W_even, V, U
NWE = 7
NV = 6
NU = 10

DEBUG = bool(int(os.environ.get("KBENCH_DEBUG", "0")))
STAGES = os.environ.get("KBENCH_STAGES", "AB")

_CACHE = {}


# --------------------------------------------------------------------------
# host-side weight packing
# --------------------------------------------------------------------------

def _pack_host(inputs):
    f = np.float32
    w1 = np.asarray(inputs["w1"], f)   # [32,1,3,3]
    w2 = np.asarray(inputs["w2"], f)   # [64,32,3,3]
    w3 = np.asarray(inputs["w3"], f)   # [32,64,3,3]
    hw_all = np.concatenate(
        [np.asarray(inputs[f"hw{i}"], f) for i in (1, 2, 3)], axis=0)

    # conv1 lhsT variants: v=0 interior (xrep rows 4g-1..4g+4),
    # v=1 top g=0 (xrep rows 0..5), v=2 bot g=127 (xrep rows 506..511)
    w1L = np.zeros((9, 6, 128), f)
    for b in range(3):
        for j in range(6):
            for q in range(4):
                for v, aa in ((0, j - q), (1, j - q + 1), (2, j - q - 1)):
                    if 0 <= aa <= 2:
                        w1L[3 * v + b, j, 32 * q:32 * q + 32] = \
                            w1[:, 0, aa, b]

    w2L = np.zeros((3, 128, 128), f)
    for b in range(3):
        for j in range(4):
            for u in range(2):
                a = j - u
                if 0 <= a <= 2:
                    w2L[b, 32 * j:32 * j + 32, 64 * u:64 * u + 64] = \
                        w2[:, :, a, b].T

    w3L = np.zeros((9, 128, 128), f)   # index 3c+b
    for c in range(3):
        for b in range(3):
            for j in range(2):
                for q in range(4):
                    a = 2 * c + j - q
                    if 0 <= a <= 2:
                        w3L[3 * c + b, 64 * j:64 * j + 64,
                            32 * q:32 * q + 32] = w3[:, :, a, b].T

    hwL = np.zeros((9, 128, 96), f)    # index 3c+b
    for c in range(3):
        for b in range(3):
            for j in range(4):
                for r in range(8):
                    a = 4 * c + j - r
                    if 0 <= a <= 2:
                        hwL[3 * c + b, 32 * j:32 * j + 32,
                            12 * r:12 * r + 12] = hw_all[:, :, a, b].T

    hwE = np.zeros((3, 64, 12), f)
    for b in range(3):
        for j in range(2):
            hwE[b, 32 * j:32 * j + 32, :] = hw_all[:, :, j + 1, b].T

    hwB = np.zeros((6, 128, 12 * NQB), f)   # index 3c+b, c=0,1
    base0 = H - 8
    for c in range(2):
        for b in range(3):
            for j in range(4):
                for q in range(NQB):
                    a = (base0 + 4 * c + j) - (H - NQB + q) + 1
                    if 0 <= a <= 2:
                        hwB[3 * c + b, 32 * j:32 * j + 32,
                            12 * q:12 * q + 12] = hw_all[:, :, a, b].T

    sel = np.zeros((96, 12), f)
    for r in range(8):
        for c in range(12):
            sel[12 * r + c, c] = 1.0

    def blockdiag(ws):
        m = np.zeros((12, 12), f)
        for i, wca in enumerate(ws):
            m[4 * i:4 * i + 4, 4 * i:4 * i + 4] = wca[:, :, 0, 0].T
        return m

    caA = blockdiag([np.asarray(inputs[f"ca{i}a"], f) for i in (1, 2, 3)]) \
        / NPIX
    caB = blockdiag([np.asarray(inputs[f"ca{i}b"], f) for i in (1, 2, 3)])

    # banded along-H blur matrices (same as baseline)
    ks = (5, 15, 25)
    bandH = np.zeros((3, 4, 3, 128, 128), f)
    for kidx, k in enumerate(ks):
        c = (k - 1) // 2
        inv = 1.0 / (k * k)
        for t in range(4):
            for relidx, rel in enumerate((-1, 0, 1)):
                tp = t + rel
                if tp < 0 or tp > 3:
                    continue
                ii = np.arange(128)[:, None] + 128 * tp
                jj = np.arange(128)[None, :] + 128 * t
                bandH[kidx, t, relidx][np.abs(ii - jj) <= c] = inv
    bandP = np.ascontiguousarray(
        np.transpose(bandH, (3, 0, 1, 2, 4)).reshape(128, 36 * 128))

    b1 = np.asarray(inputs["b1"], f)
    b2 = np.asarray(inputs["b2"], f)
    b3 = np.asarray(inputs["b3"], f)
    hb = np.concatenate([np.asarray(inputs[f"hb{i}"], f)
                         for i in (1, 2, 3)])
    return dict(
        w1L=w1L, w2L=w2L, w3L=w3L, hwL=hwL, hwE=hwE, hwB=hwB,
        sel=sel, caA=caA, caB=caB, bandP=bandP,
        b1x4=np.tile(b1, 4).reshape(128, 1),
        b2x2=np.tile(b2, 2).reshape(128, 1),
        b3x4=np.tile(b3, 4).reshape(128, 1),
        hbx8=np.tile(hb, 8).reshape(96, 1),
        hbx7=np.tile(hb, NQB).reshape(84, 1),
        hb=hb.reshape(12, 1),
    )


# --------------------------------------------------------------------------
# kernel build
# --------------------------------------------------------------------------

def _build(alpha1, alpha2, alpha3, debug=False, loop_reps=0, stages="AB"):
    import concourse.bacc as bacc
    import concourse.mybir as mybir
    import concourse.tile as tile

    dt = mybir.dt
    AFT = mybir.ActivationFunctionType
    f32 = dt.float32
    f32r = dt.float32r

    nc = bacc.Bacc("TRN2", target_bir_lowering=False, debug=False,
                   num_devices=NCORES)

    # ---- I/O ----
    xb = nc.dram_tensor("xb", [H, W], f32, kind="ExternalInput")
    w1L_d = nc.dram_tensor("w1L", [9, 6, 128], f32, kind="ExternalInput")
    w2L_d = nc.dram_tensor("w2L", [3, 128, 128], f32, kind="ExternalInput")
    w3L_d = nc.dram_tensor("w3L", [9, 128, 128], f32, kind="ExternalInput")
    hwL_d = nc.dram_tensor("hwL", [9, 128, 96], f32, kind="ExternalInput")
    hwE_d = nc.dram_tensor("hwE", [3, 64, 12], f32, kind="ExternalInput")
    hwB_d = nc.dram_tensor("hwB", [6, 128, 12 * NQB], f32,
                           kind="ExternalInput")
    sel_d = nc.dram_tensor("sel", [96, 12], f32, kind="ExternalInput")
    caA_d = nc.dram_tensor("caA", [12, 12], f32, kind="ExternalInput")
    caB_d = nc.dram_tensor("caB", [12, 12], f32, kind="ExternalInput")
    bandP_d = nc.dram_tensor("bandP", [128, 36 * 128], f32,
                             kind="ExternalInput")
    b1x4_d = nc.dram_tensor("b1x4", [128, 1], f32, kind="ExternalInput")
    b2x2_d = nc.dram_tensor("b2x2", [128, 1], f32, kind="ExternalInput")
    b3x4_d = nc.dram_tensor("b3x4", [128, 1], f32, kind="ExternalInput")
    hbx8_d = nc.dram_tensor("hbx8", [96, 1], f32, kind="ExternalInput")
    hbx7_d = nc.dram_tensor("hbx7", [84, 1], f32, kind="ExternalInput")
    hb_d = nc.dram_tensor("hb", [12, 1], f32, kind="ExternalInput")

    outb = nc.dram_tensor("outb", [H, W], f32, kind="ExternalOutput")

    dbg = {}
    if debug:
        for name, shape in (("d_y", [H, 12, W]), ("d_g", [12, 1]),
                            ("d_h", [12, H, W]),
                            ("d_out1", [H, W]), ("d_out2", [H, W])):
            dbg[name] = nc.dram_tensor(name, shape, f32,
                                       kind="ExternalOutput")

    with tile.TileContext(nc) as tc:
        with (
            tc.tile_pool(name="dram", bufs=1, space="DRAM") as dpool,
            tc.tile_pool(name="wsb", bufs=1) as wsb,
        ):
            y_dram = dpool.tile([H, 12, W], f32)

            # persistent weight/bias tiles
            w1sb = wsb.tile([6, 9 * 128], f32r)
            w2sb = wsb.tile([128, 3 * 128], f32r)
            w3sb = wsb.tile([128, 9 * 128], f32r)
            hwsb = wsb.tile([128, 9 * 96], f32r)
            hwEsb = wsb.tile([64, 3 * 12], f32r)
            hwBsb = wsb.tile([128, 6 * 12 * NQB], f32r)
            selsb = wsb.tile([96, 12], f32)
            caAsb = wsb.tile([12, 12], f32)
            caBsb = wsb.tile([12, 12], f32)
            bandsb = wsb.tile([128, 36 * 128], f32r)
            b1sb = wsb.tile([128, 1], f32)
            b2sb = wsb.tile([128, 1], f32)
            b3sb = wsb.tile([128, 1], f32)
            hb8sb = wsb.tile([96, 1], f32)
            hb7sb = wsb.tile([84, 1], f32)
            hbsb = wsb.tile([12, 1], f32)
            onesb = wsb.tile([1, 128], f32)
            accums = wsb.tile([96, NGH + 3], f32)

            with tc.tile_pool(name="wstg", bufs=1) as wstg:
                w1f = wstg.tile([6, 9 * 128], f32)
                w2f = wstg.tile([128, 3 * 128], f32)
                w3f = wstg.tile([128, 9 * 128], f32)
                hwf = wstg.tile([128, 9 * 96], f32)
                hwEf = wstg.tile([64, 3 * 12], f32)
                hwBf = wstg.tile([128, 6 * 12 * NQB], f32)
                bandf = wstg.tile([128, 36 * 128], f32)
                for b in range(3):
                    nc.sync.dma_start(w2f[:, 128 * b:128 * (b + 1)],
                                      w2L_d[b])
                    nc.sync.dma_start(hwEf[:, 12 * b:12 * (b + 1)],
                                      hwE_d[b])
                for i in range(9):
                    nc.sync.dma_start(w1f[:, 128 * i:128 * (i + 1)],
                                      w1L_d[i])
                    nc.sync.dma_start(w3f[:, 128 * i:128 * (i + 1)],
                                      w3L_d[i])
                    nc.sync.dma_start(hwf[:, 96 * i:96 * (i + 1)],
                                      hwL_d[i])
                for i in range(6):
                    nc.sync.dma_start(hwBf[:, 84 * i:84 * (i + 1)],
                                      hwB_d[i])
                nc.sync.dma_start(bandf[:], bandP_d[:])
                nc.vector.tensor_copy(w1sb[:], w1f[:])
                nc.vector.tensor_copy(w2sb[:], w2f[:])
                nc.vector.tensor_copy(w3sb[:], w3f[:])
                nc.vector.tensor_copy(hwsb[:], hwf[:])
                nc.vector.tensor_copy(hwEsb[:], hwEf[:])
                nc.vector.tensor_copy(hwBsb[:], hwBf[:])
                nc.vector.tensor_copy(bandsb[:], bandf[:])
            nc.sync.dma_start(selsb[:], sel_d[:])
            nc.sync.dma_start(caAsb[:], caA_d[:])
            nc.sync.dma_start(caBsb[:], caB_d[:])
            nc.sync.dma_start(b1sb[:], b1x4_d[:])
            nc.sync.dma_start(b2sb[:], b2x2_d[:])
            nc.sync.dma_start(b3sb[:], b3x4_d[:])
            nc.sync.dma_start(hb8sb[:], hbx8_d[:])
            nc.sync.dma_start(hb7sb[:], hbx7_d[:])
            nc.sync.dma_start(hbsb[:], hb_d[:])
            nc.vector.memset(onesb[:], 1.0)
            nc.vector.memset(accums[:], 0.0)

            def phase_a():
                with (
                    tc.tile_pool(name="abuf", bufs=1) as abuf,
                    tc.tile_pool(name="xrp", bufs=5) as xrp,
                    tc.tile_pool(name="ystg", bufs=2) as ystg,
                    tc.tile_pool(name="ps1", bufs=2, space="PSUM") as ps1,
                    tc.tile_pool(name="ps2", bufs=3, space="PSUM") as ps2,
                    tc.tile_pool(name="ps3", bufs=2, space="PSUM") as ps3,
                    tc.tile_pool(name="psH", bufs=1, space="PSUM") as psH,
                ):
                    x_sb = abuf.tile([128, 4, PW], f32r, name="x_sb")
                    WO = abuf.tile([128, NWO, PW], f32r, name="WO")
                    WE = abuf.tile([128, NWE, PW], f32r, name="WE")
                    Vt = abuf.tile([128, NV, PW], f32r, name="Vt")
                    Ut = abuf.tile([128, NU, PW], f32r, name="Ut")

                    # x load + round to f32r + pads
                    xstg = abuf.tile([128, 4, 512], f32, name="xstg")
                    nc.sync.dma_start(
                        xstg[:], xb[:, :].rearrange("(b p) w -> p b w",
                                                    p=128))
                    nc.vector.tensor_copy(x_sb[:, :, 1:513], xstg[:])
                    nc.vector.memset(x_sb.bitcast(f32)[:, :, 0:1], 0.0)
                    nc.vector.memset(x_sb.bitcast(f32)[:, :, 513:514], 0.0)
                    # column pads of line buffers (stay zero forever)
                    for t_ in (WO, WE, Vt, Ut):
                        nc.vector.memset(t_.bitcast(f32)[:, :, 0:1], 0.0)
                        nc.vector.memset(t_.bitcast(f32)[:, :, 513:514], 0.0)
                    # boundary zeros
                    nc.vector.memset(WE.bitcast(f32)[0:64, 0, :], 0.0)
                    nc.vector.memset(Vt.bitcast(f32)[0:64, 0, :], 0.0)

                    xr_tiles = {}

                    def xrep_fetch(g):
                        # rows copied into xr partitions 0..5 via DMA
                        # (no partition-base restriction on DMA)
                        xr = xrp.tile([6, PW], f32r, tag="xrep",
                                      name="xrepT")
                        if g == 0:
                            base = 0        # rows 0..5, lhsT variant v=1
                        elif g == 127:
                            base = 506      # rows 506..511, variant v=2
                        else:
                            base = 4 * g - 1
                        r = base
                        while r <= base + 5:
                            blk = r // 128
                            rend = min(base + 5, blk * 128 + 127)
                            j0 = r - base
                            n = rend - r + 1
                            nc.sync.dma_start(
                                xr[j0:j0 + n, :],
                                x_sb[r % 128:r % 128 + n, blk, :])
                            r = rend + 1
                        xr_tiles[g] = xr

                    def conv1(g):
                        xr = xr_tiles.pop(g)
                        v = 1 if g == 0 else (2 if g == 127 else 0)
                        psum1 = ps1.tile([128, 512], f32, tag="ps1",
                                         name="ps1T")
                        for b in range(3):
                            nc.tensor.matmul(
                                psum1[:],
                                w1sb[0:6, 128 * (3 * v + b):
                                     128 * (3 * v + b) + 128],
                                xr[0:6, b:b + 512],
                                start=(b == 0), stop=(b == 2))
                        nc.scalar.activation(
                            WO[:, g % NWO, 1:513], psum1[:], AFT.Prelu,
                            bias=b1sb[:], scale=1.0, alpha=alpha1)
                        nc.gpsimd.tensor_copy(WE[64:128, g % NWE, :],
                                              WO[0:64, g % NWO, :])
                        nc.gpsimd.tensor_copy(WE[0:64, (g + 1) % NWE, :],
                                              WO[64:128, g % NWO, :])

                    def conv2(t):
                        if t % 2 == 0:
                            Wx = WE[:, (t // 2) % NWE, :]
                        else:
                            Wx = WO[:, (t // 2) % NWO, :]
                        psum2 = ps2.tile([128, 512], f32, tag="ps2",
                                         name="ps2T")
                        for b in range(3):
                            nc.tensor.matmul(
                                psum2[:],
                                w2sb[:, 128 * b:128 * b + 128],
                                Wx[:, b:b + 512],
                                start=(b == 0), stop=(b == 2))
                        if t == 0:
                            nc.scalar.activation(
                                Vt[64:128, 0, 1:513], psum2[64:128, :],
                                AFT.Prelu, bias=b2sb[64:128], scale=1.0,
                                alpha=alpha2)
                        elif t == NT - 1:
                            nc.scalar.activation(
                                Vt[0:64, t % NV, 1:513], psum2[0:64, :],
                                AFT.Prelu, bias=b2sb[0:64], scale=1.0,
                                alpha=alpha2)
                        else:
                            nc.scalar.activation(
                                Vt[:, t % NV, 1:513], psum2[:], AFT.Prelu,
                                bias=b2sb[:], scale=1.0, alpha=alpha2)

                    def conv3(h):
                        psum3 = ps3.tile([128, 512], f32, tag="ps3",
                                         name="ps3T")
                        for c in range(3):
                            for b in range(3):
                                nc.tensor.matmul(
                                    psum3[:],
                                    w3sb[:, 128 * (3 * c + b):
                                         128 * (3 * c + b) + 128],
                                    Vt[:, (2 * h + c) % NV, b:b + 512],
                                    start=(c == 0 and b == 0),
                                    stop=(c == 2 and b == 2))
                        nc.scalar.activation(
                            Ut[:, h % NU, 1:513], psum3[:], AFT.Prelu,
                            bias=b3sb[:], scale=1.0, alpha=alpha3)

                    def heads(i):
                        psumH = psH.tile([96, 512], f32, tag="psH",
                                         name="psHT")
                        for c in range(3):
                            for b in range(3):
                                nc.tensor.matmul(
                                    psumH[:],
                                    hwsb[:, 96 * (3 * c + b):
                                         96 * (3 * c + b) + 96],
                                    Ut[:, (2 * i + c) % NU, b:b + 512],
                                    start=(c == 0 and b == 0),
                                    stop=(c == 2 and b == 2))
                        stg = ystg.tile([96, 512], f32, tag="ystg",
                                        name="ystgT")
                        nc.scalar.activation(
                            stg[:], psumH[:], AFT.Prelu, bias=hb8sb[:],
                            scale=1.0, alpha=1.0,
                            accum_out=accums[:, i:i + 1])
                        nc.sync.dma_start(
                            y_dram[8 * i + 1:8 * i + 9, :, :], stg[:])

                    def heads_top():
                        psE = psH.tile([12, 512], f32, tag="psH",
                                       name="psET")
                        for b in range(3):
                            nc.tensor.matmul(
                                psE[:], hwEsb[:, 12 * b:12 * b + 12],
                                Ut[0:64, 0, b:b + 512],
                                start=(b == 0), stop=(b == 2))
                        stg = ystg.tile([12, 512], f32, tag="ystgE",
                                        name="ystgET", bufs=1)
                        nc.scalar.activation(
                            stg[:], psE[:], AFT.Prelu, bias=hbsb[:],
                            scale=1.0, alpha=1.0,
                            accum_out=accums[0:12, NGH:NGH + 1])
                        nc.sync.dma_start(y_dram[0:1, :, :], stg[:])

                    def heads_bot():
                        psB = psH.tile([84, 512], f32, tag="psH",
                                       name="psBT")
                        for c in range(2):
                            for b in range(3):
                                nc.tensor.matmul(
                                    psB[:],
                                    hwBsb[:, 84 * (3 * c + b):
                                          84 * (3 * c + b) + 84],
                                    Ut[:, (126 + c) % NU, b:b + 512],
                                    start=(c == 0 and b == 0),
                                    stop=(c == 1 and b == 2))
                        stg = ystg.tile([84, 512], f32, tag="ystgB",
                                        name="ystgBT", bufs=1)
                        nc.scalar.activation(
                            stg[:], psB[:], AFT.Prelu, bias=hb7sb[:],
                            scale=1.0, alpha=1.0,
                            accum_out=accums[0:84, NGH + 1:NGH + 2])
                        nc.sync.dma_start(y_dram[H - NQB:H, :, :], stg[:])

                    for g in range(134):
                        if g == 0:
                            xrep_fetch(0)
                            xrep_fetch(1)
                            xrep_fetch(2)
                        if g + 3 <= 127:
                            xrep_fetch(g + 3)
                        if g <= 127:
                            conv1(g)
                        if g == 128:
                            nc.vector.memset(
                                WE.bitcast(f32)[64:128, 128 % NWE, :], 0.0)
                        if g == 130:
                            # virtual body2 row 512 (chunk 256 upper half)
                            nc.vector.memset(
                                Vt.bitcast(f32)[64:128, (NT - 1) % NV, :],
                                0.0)
                        if 2 <= g <= 129:
                            conv2(2 * (g - 2))
                            conv2(2 * (g - 2) + 1)
                        if g == 130:
                            conv2(NT - 1)
                        if 4 <= g <= 131:
                            conv3(g - 4)
                        if g == 5:
                            heads_top()
                        if g % 2 == 1 and 7 <= g <= 131:
                            heads((g - 7) // 2)
                        if g == 133:
                            heads_bot()

            def phase_b():
                with (
                    tc.tile_pool(name="bsm", bufs=1) as bsm,
                    tc.tile_pool(name="bps1", bufs=1, space="PSUM") as bps1,
                    tc.tile_pool(name="bps", bufs=2, space="PSUM") as bps,
                    tc.tile_pool(name="bbl", bufs=1) as bbl,
                ):
                    # ---- CA gating ----
                    tot96 = bsm.tile([96, 1], f32, name="tot96")
                    nc.vector.reduce_sum(tot96[:], accums[:],
                                         axis=mybir.AxisListType.X)
                    ps12 = bps1.tile([12, 1], f32, tag="caps", name="ps12")
                    nc.tensor.matmul(ps12[:], selsb[:], tot96[:],
                                     start=True, stop=True)
                    total = bsm.tile([12, 1], f32, name="total")
                    nc.vector.tensor_copy(total[:], ps12[:])
                    psA = bps1.tile([12, 1], f32, tag="caps", name="psA")
                    nc.tensor.matmul(psA[:], caAsb[:], total[:],
                                     start=True, stop=True)
                    trelu = bsm.tile([12, 1], f32, name="trelu")
                    nc.scalar.activation(trelu[:], psA[:], AFT.Relu)
                    psB_ = bps1.tile([12, 1], f32, tag="caps", name="psB_")
                    nc.tensor.matmul(psB_[:], caBsb[:], trelu[:],
                                     start=True, stop=True)
                    g_gate = bsm.tile([12, 1], f32, name="g_gate")
                    nc.scalar.activation(g_gate[:], psB_[:], AFT.Sigmoid)
                    if debug:
                        nc.sync.dma_start(dbg["d_g"][:], g_gate[:])
                    g_row = bsm.tile([1, 12], f32, name="g_row")
                    nc.sync.dma_start(g_row[:], g_gate[:])
                    psG = bps1.tile([128, 12], f32, tag="gbc", name="psG")
                    nc.tensor.matmul(psG[:], onesb[:], g_row[:],
                                     start=True, stop=True)
                    gbc = bsm.tile([128, 12], f32, name="gbc")
                    nc.vector.tensor_copy(gbc[:], psG[:])

                    # ---- blur planes ----
                    FW = 4 * BS  # 2240
                    u = bbl.tile([128, FW], f32r, name="u")
                    S2 = bbl.tile([128, FW], f32r, name="S2")
                    S4 = bbl.tile([128, FW], f32r, name="S4")
                    S8 = bbl.tile([128, FW], f32r, name="S8")
                    S16 = bbl.tile([128, FW], f32r, name="S16")
                    S5 = bbl.tile([128, FW], f32r, name="S5")
                    S15 = bbl.tile([128, FW], f32r, name="S15")
                    S25 = bbl.tile([128, FW], f32r, name="S25")
                    unext = bbl.tile([128, FW], f32r, name="unext")
                    t1 = bbl.tile([128, 512], f32, name="t1")
                    t2 = bbl.tile([128, 512], f32, name="t2")
                    ostg = bbl.tile([128, 4, 512], f32, name="ostg")
                    nc.vector.memset(u[:].bitcast(f32), 0.0)
                    nc.vector.memset(unext[:].bitcast(f32), 0.0)

                    xt2 = bsm.tile([128, 4, 512], f32, name="xt2")
                    nc.sync.dma_start(
                        xt2[:], xb[:, :].rearrange("(b p) w -> p b w", p=128))
                    uview = u[:].rearrange("p (b w) -> p b w", b=4)
                    nc.vector.tensor_copy(uview[:, :, DOFF:DOFF + 512],
                                          xt2[:])

                    ep = [bsm.tile([128, 4, 512], f32, tag=f"exp{c}",
                                   name=f"ep{c}")
                          for c in range(4)]
                    yt = bsm.tile([128, 4, 512], f32, name="yt")
                    tsum = bsm.tile([128, 4, 512], f32, name="tsum")

                    cs = {5: 2, 15: 7, 25: 12}
                    ucur, unxt = u, unext
                    for stage in range(3):
                        for c in range(4):
                            cg = 4 * stage + c
                            nc.sync.dma_start(
                                yt[:],
                                y_dram[:, cg, :].rearrange(
                                    "(b p) w -> p b w", p=128))
                            nc.scalar.activation(ep[c][:], yt[:], AFT.Exp,
                                                 scale=gbc[:, cg:cg + 1])
                        nc.vector.tensor_add(tsum[:], ep[0][:], ep[1][:])
                        nc.vector.tensor_add(tsum[:], tsum[:], ep[2][:])
                        nc.vector.tensor_add(tsum[:], tsum[:], ep[3][:])
                        nc.vector.reciprocal(tsum[:], tsum[:])
                        for c in range(4):
                            nc.vector.tensor_mul(ep[c][:], ep[c][:],
                                                 tsum[:])
                        if debug:
                            for c in range(4):
                                nc.sync.dma_start(
                                    dbg["d_h"][4 * stage + c].rearrange(
                                        "(b p) w -> p b w", p=128),
                                    ep[c][:])

                        wv = FW - 24
                        nc.vector.tensor_add(S2[:, 0:wv], ucur[:, 0:wv],
                                             ucur[:, 1:1 + wv])
                        nc.vector.tensor_add(S4[:, 0:wv], S2[:, 0:wv],
                                             S2[:, 2:2 + wv])
                        nc.vector.tensor_add(S8[:, 0:wv], S4[:, 0:wv],
                                             S4[:, 4:4 + wv])
                        nc.vector.tensor_add(S16[:, 0:wv], S8[:, 0:wv],
                                             S8[:, 8:8 + wv])
                        nc.vector.tensor_add(S5[:, 0:wv], S4[:, 0:wv],
                                             ucur[:, 4:4 + wv])
                        nc.vector.tensor_sub(S15[:, 0:wv], S16[:, 0:wv],
                                             ucur[:, 15:15 + wv])
                        nc.vector.tensor_add(S25[:, 0:wv], S16[:, 0:wv],
                                             S8[:, 16:16 + wv])
                        nc.vector.tensor_add(S25[:, 0:wv], S25[:, 0:wv],
                                             ucur[:, 24:24 + wv])

                        Sk = {5: S5, 15: S15, 25: S25}
                        for t in range(4):
                            pk = {}
                            for kidx, k in enumerate((5, 15, 25)):
                                ps = bps.tile([128, 512], f32,
                                              tag=f"blur{kidx}",
                                              name=f"blur{kidx}")
                                rels = [r for r in (-1, 0, 1)
                                        if 0 <= t + r <= 3]
                                for ri, rel in enumerate(rels):
                                    idx = kidx * 12 + t * 3 + (rel + 1)
                                    off = (t + rel) * BS + DOFF - cs[k]
                                    nc.tensor.matmul(
                                        ps[:],
                                        bandsb[:, idx * 128:(idx + 1) * 128],
                                        Sk[k][:, off:off + 512],
                                        start=(ri == 0),
                                        stop=(ri == len(rels) - 1))
                                pk[k] = ps
                            ub = ucur[:, t * BS + DOFF:t * BS + DOFF + 512]
                            nc.vector.tensor_mul(t1[:], ep[0][:, t, :], ub)
                            nc.vector.tensor_mul(t2[:], ep[1][:, t, :],
                                                 pk[5][:])
                            nc.vector.tensor_add(t1[:], t1[:], t2[:])
                            nc.vector.tensor_mul(t2[:], ep[2][:, t, :],
                                                 pk[15][:])
                            nc.vector.tensor_add(t1[:], t1[:], t2[:])
                            nc.vector.tensor_mul(t2[:], ep[3][:, t, :],
                                                 pk[25][:])
                            if stage < 2:
                                nc.vector.tensor_add(
                                    unxt[:, t * BS + DOFF:
                                         t * BS + DOFF + 512],
                                    t1[:], t2[:])
                            else:
                                nc.vector.tensor_add(ostg[:, t, :], t1[:],
                                                     t2[:])
                        if stage < 2:
                            ucur, unxt = unxt, ucur
                            if debug:
                                dv = ucur[:].rearrange("p (b w) -> p b w",
                                                       b=4)
                                ds = bbl.tile([128, 4, 512], f32,
                                              tag="dbgo", name="dbgo")
                                nc.vector.tensor_copy(
                                    ds[:], dv[:, :, DOFF:DOFF + 512])
                                nc.sync.dma_start(
                                    dbg[f"d_out{stage + 1}"][:, :].rearrange(
                                        "(b p) w -> p b w", p=128), ds[:])

                    nc.sync.dma_start(
                        outb[:, :].rearrange("(b p) w -> p b w", p=128),
                        ostg[:])

            def phases():
                if "A" in stages:
                    phase_a()
                if debug:
                    nc.sync.dma_start(dbg["d_y"][:], y_dram[:])
                if "B" in stages:
                    phase_b()

            if loop_reps:
                with tc.For_i(0, loop_reps, 1):
                    phases()
            else:
                phases()

    nc.compile()
    return nc


# --------------------------------------------------------------------------
# PJRT runner (unchanged from baseline)
# --------------------------------------------------------------------------

class _Runner:
    def __init__(self, nc):
        import jax
        import concourse.mybir as mybir
        from concourse import bass2jax
        from jax.sharding import Mesh, PartitionSpec
        from jax.experimental.shard_map import shard_map

        bass2jax.install_neuronx_cc_hook()
        self.nc = nc
        in_names, out_names, out_avals, zero_outs = [], [], [], []
        partition_name = (nc.partition_id_tensor.name
                          if nc.partition_id_tensor else None)
        for alloc in nc.m.functions[0].allocations:
            if not isinstance(alloc, mybir.MemoryLocationSet):
                continue
            name = alloc.memorylocations[0].name
            if alloc.kind == "ExternalInput":
                if name != partition_name:
                    in_names.append(name)
            elif alloc.kind == "ExternalOutput":
                out_names.append(name)
                shape = tuple(alloc.tensor_shape)
                dtype = mybir.dt.np(alloc.dtype)
                out_avals.append(jax.core.ShapedArray(shape, dtype))
                zero_outs.append(np.zeros(shape, dtype))
        self.in_names = list(in_names)
        self.out_names = out_names
        self.out_avals = out_avals
        self.zero_outs = zero_outs
        n_params = len(in_names)
        n_outs = len(out_names)
        all_names = in_names + out_names
        if partition_name is not None:
            all_names.append(partition_name)

        def _body(*args):
            operands = list(args)
            if partition_name is not None:
                operands.append(bass2jax.partition_id_tensor())
            outs = bass2jax._bass_exec_p.bind(
                *operands,
                out_avals=tuple(out_avals),
                in_names=tuple(all_names),
                out_names=tuple(out_names),
                lowering_input_output_aliases=(),
                sim_require_finite=True,
                sim_require_nnan=True,
                nc=nc,
            )
            return tuple(outs)

        devices = jax.devices()[:NCORES]
        mesh = Mesh(np.asarray(devices), ("core",))
        in_specs = (PartitionSpec("core"),) * (n_params + n_outs)
        out_specs = (PartitionSpec("core"),) * n_outs
        self.sharded = jax.jit(
            shard_map(_body, mesh=mesh, in_specs=in_specs,
                      out_specs=out_specs, check_rep=False),
            keep_unused=True,
        )

    def concat_inputs(self, in_maps):
        return [
            np.concatenate([np.asarray(in_maps[c][nm])
                            for c in range(NCORES)], axis=0)
            for nm in self.in_names
        ]

    def concat_zeros(self):
        return [np.zeros((NCORES * z.shape[0], *z.shape[1:]), z.dtype)
                for z in self.zero_outs]

    def __call__(self, in_maps):
        out_arrs = self.sharded(*self.concat_inputs(in_maps),
                                *self.concat_zeros())
        return [
            {nm: np.asarray(out_arrs[i]).reshape(
                NCORES, *self.out_avals[i].shape)[c]
             for i, nm in enumerate(self.out_names)}
            for c in range(NCORES)
        ]


def _get_runner(alpha1, alpha2, alpha3, loop_reps=0, stages=None):
    if stages is None:
        stages = STAGES
    key = ("runner", alpha1, alpha2, alpha3, DEBUG, loop_reps, stages)
    if key not in _CACHE:
        key_nc = (alpha1, alpha2, alpha3, DEBUG, loop_reps, stages)
        if key_nc not in _CACHE:
            _CACHE[key_nc] = _build(alpha1, alpha2, alpha3, debug=DEBUG,
                                    loop_reps=loop_reps, stages=stages)
        _CACHE[key] = _Runner(_CACHE[key_nc])
    return _CACHE[key]


def make_in_maps(inputs):
    x = np.asarray(inputs["x"], np.float32)   # [8,1,512,512]
    packed = _pack_host(inputs)
    in_maps = []
    for i in range(NCORES):
        m = {"xb": np.ascontiguousarray(x[i, 0])}
        m.update(packed)
        in_maps.append(m)
    return in_maps


def kernel(**inputs):
    runner = _get_runner(float(inputs["a1"]), float(inputs["a2"]),
                         float(inputs["a3"]))
    results = runner(make_in_maps(inputs))
    out = np.stack([results[i]["outb"] for i in range(NCORES)])
    globals()["_LAST_RESULTS"] = results
    return out.reshape(8, 1, H, W).astype(np.float32)
